# revision 53
# baseline (speedup 1.0000x reference)
"""Trainium2 Bass kernel for the HHGLCM few-shot EMD head (v3).

Per NeuronCore (data-parallel over queries, 8 cores): query shard
[256, 640, 5, 5] + full proto [64, 640, 5, 5], two 128-query tiles.

Measured-rate design notes (TRN2):
  - DVE: packed bf16 tensor_tensor runs 2x (0.52 ns/col); stt and any
    stride-0/strided operand run 1x or worse; tensor_reduce always 1.04.
  - Replications (u/v/1-a over the 5x5 cell) are folded into scalar-engine
    activations reading stride-0 broadcast views and writing packed bf16,
    so every elementwise multiply on DVE is packed 2x.
  - Pooling: shared-partial column adds (strided, split DVE/GpSimd) then
    packed bf16 row adds.
  - PE: bf16 matmuls, 25 [128,128] transposes per tile; proto rhs tensors
    are duplicated at partitions 64:128 so both 64-channel runs of a
    transposed chunk can be used as lhsT directly.
  - Sinkhorn: 3 scaling iterations (validated ~7e-3 rel l2 vs the
    100-iteration fp32 reference, gate is 2e-2).
"""

from contextlib import ExitStack

import numpy as np

import concourse.bacc as bacc
import concourse.mybir as mybir
from concourse import masks
from concourse.tile import TileContext

F32 = mybir.dt.float32
BF16 = mybir.dt.bfloat16
AX = mybir.AxisListType
ALU = mybir.AluOpType
ACTF = mybir.ActivationFunctionType

N_CORES = 8
NQ = 2048
QPC = NQ // N_CORES
QT = 128
NT = QPC // QT
C = 640
W = 64
P = 5
S = 25
EPS = 0.05
TEMP = 12.5
ITERS = 3
EXP_SCALE = 1.0 / EPS
EXP_BIAS = -1.0 / EPS + float(np.log(0.2))
FINAL_SCALE = (TEMP / P) / 0.2
PATCH_W2 = [1.0 / 81, 1.0 / 81, 1.0 / 256, 1.0 / 81, 1.0 / 81]

NRUN = 10
RC = 64
CQ = C // 4  # 160 channels per DMA quarter
WM = 65      # wm rhs width: 64 ways + 1 ones column (msum)


def _col_stage(nc, cws, x, cn, c0, plan):
    """Column-window sums for channel range c0:c0+cn. x: [p,(cn,5,5)] fp32.
    cwa=cols0:3, cwb=cols1:5, cwc=cols2:5, t5=x3+x4 helper. plan: list of 5
    engines for the 5 adds."""
    cwa, cwb, cwc, t5 = cws
    xs = [x[:, :, :, k] for k in range(5)]

    def dst(cw):
        return cw.rearrange("p (r c) -> p c r", r=P)[:, c0 : c0 + cn, :]

    t5v = t5[:, 0 : cn * P].rearrange("p (c r) -> p c r", r=P)
    plan[0].tensor_add(dst(cwa), xs[0], xs[1])
    plan[1].tensor_add(dst(cwa), dst(cwa), xs[2])
    plan[2].tensor_add(t5v, xs[3], xs[4])
    plan[3].tensor_add(dst(cwc), t5v, xs[2])
    plan[4].tensor_add(dst(cwb), dst(cwc), xs[1])


def _row_stage(nc, qf, cwa, cwb, cwc, scr, width=C):
    """Row-window sums -> qf [q,(5p,width)] bf16, packed 2x adds."""
    C_ = width

    def r(cw, i):
        return cw[:, i * C_ : (i + 1) * C_]

    def qp(i):
        return qf[:, i * C_ : (i + 1) * C_]

    va = nc.vector.tensor_add
    ga = nc.gpsimd.tensor_add
    t0 = scr[:, 0:C_]
    t1 = scr[:, C_ : 2 * C_]
    va(t0, r(cwa, 0), r(cwa, 1))
    va(qp(0), t0, r(cwa, 2))          # lt
    va(t1, r(cwa, 3), r(cwa, 4))
    va(qp(1), t1, r(cwa, 2))          # rt
    ga(t0, r(cwb, 1), r(cwb, 2))
    ga(t1, r(cwb, 3), r(cwb, 4))
    ga(qp(2), t0, t1)                 # mid
    va(t0, r(cwc, 0), r(cwc, 1))
    va(qp(3), t0, r(cwc, 2))          # lb
    va(t1, r(cwc, 3), r(cwc, 4))
    va(qp(4), t1, r(cwc, 2))          # rb


def build_bass():
    nc = bacc.Bacc()
    query = nc.declare_dram_parameter("query", [QPC, C, 5, 5], F32, isOutput=False)
    proto = nc.declare_dram_parameter("proto", [1, W, C, 5, 5], F32, isOutput=False)
    out = nc.declare_dram_parameter("out", [QPC, W], F32, isOutput=True)

    ctx = ExitStack()
    with ctx:
        tc = ctx.enter_context(TileContext(nc))
        _build_body(ctx, tc, nc, query, proto, out)
    nc.finalize()
    return nc


def _build_body(ctx, tc, nc, query, proto, out):
    const_pool = ctx.enter_context(tc.tile_pool(name="const", bufs=1))
    ident_bf = const_pool.tile([128, 128], BF16)
    masks.make_identity(nc, ident_bf[:])
    ebias = const_pool.tile([128, 1], F32)
    nc.vector.memset(ebias[:], EXP_BIAS)

    # persistent proto products (channel-partition, 64 rows)
    ppers = ctx.enter_context(tc.tile_pool(name="ppers", bufs=1))
    pn_bf = ppers.tile([RC, NRUN * W * P], BF16)   # (run, w, j) centered+normed
    wm_bf = ppers.tile([RC, NRUN * P * WM], BF16)  # (run, p, 64w+1)
    spn_b = ppers.tile([128, W * P], F32)

    CH = C // 2  # channels per proto row
    PQ = CH // 4  # 80-channel proto load chunks

    def emit_preamble_load():
        # proto rows (w, chalf): row 2w+ch holds channels [ch*320, +320);
        # loaded in 4 column chunks, split across the two HWDGE queues
        pv = proto[0].rearrange("w (ch c) h v -> (w ch) (c h v)", ch=2)
        emit_preamble_load.chunks = []
        for qtr in range(4):
            pch = _pload.tile([128, PQ * S], F32, tag="pch")
            eng = nc.sync if qtr < 2 else nc.scalar
            eng.dma_start(
                out=pch[:], in_=pv[:, qtr * PQ * S : (qtr + 1) * PQ * S]
            )
            emit_preamble_load.chunks.append(pch)

    def emit_preamble_compute():
        pcw = [
            _ppool.tile([128, P * CH], BF16, name=f"pcw{i}", tag=f"pcw{i}")
            for i in range(3)
        ]
        pt5 = _ppool.tile([128, PQ * P], BF16)
        g, v = nc.gpsimd, nc.vector
        plans = [[g, v, g, g, v], [v, g, g, v, g], [g, v, g, g, v], [v, g, g, v, g]]
        for qtr in range(4):
            pch = emit_preamble_load.chunks[qtr]
            _col_stage(
                nc, (*pcw, pt5),
                pch.rearrange("p (c h v) -> p c h v", h=5, v=5),
                PQ, qtr * PQ, plans[qtr],
            )
        pfsum = _ppool.tile([128, P * CH], BF16)
        pscr2 = _ppool.tile([128, 2 * CH], BF16)
        _row_stage(nc, pfsum, *pcw, pscr2, width=CH)

        # transpose to pT [64c, (run, p, w)] bf16 + scaled wm_bf; copies
        # split across Act/DVE/GpSimd
        pT = _ppool.tile([RC, NRUN * P * W], BF16)
        for idx in range(25):
            pi, cs = idx // 5, idx % 5
            pt_ps = _ppsA.tile([RC, 128], BF16, tag="ptps")
            nc.tensor.transpose(
                pt_ps[:],
                pfsum[:, pi * CH + cs * RC : pi * CH + (cs + 1) * RC],
                ident_bf[:],
            )
            for ch in range(2):
                run = ch * 5 + cs
                dst = slice((run * P + pi) * W, (run * P + pi + 1) * W)
                src = pt_ps[:, ch : ch + 127 : 2]
                if (idx + ch) % 2 == 0:
                    nc.scalar.copy(out=pT[:, dst], in_=src)
                    nc.vector.tensor_scalar_mul(
                        wm_bf[:, (run * P + pi) * WM : (run * P + pi) * WM + W],
                        src, PATCH_W2[pi],
                    )
                else:
                    nc.vector.tensor_copy(pT[:, dst], src)
                    nc.scalar.activation(
                        wm_bf[:, (run * P + pi) * WM : (run * P + pi) * WM + W],
                        src, ACTF.Copy, scale=PATCH_W2[pi],
                    )
        wmv = wm_bf.rearrange("c (g e) -> c g e", e=WM)
        nc.vector.memset(wmv[:, :, W : W + 1], 1.0)

        # per-(p,w) channel sums / square-sums via ones-matmuls
        ones64 = _ppool.tile([RC, 1], BF16)
        nc.vector.memset(ones64[:], 1.0)
        pm_ps = _ppsB.tile([1, W * P], F32, tag="pmps")
        psq_ps = _ppsB.tile([1, W * P], F32, tag="pmps")
        psqt = _ppool.tile([RC, W * P], BF16)
        for r in range(NRUN):
            sl = slice(r * W * P, (r + 1) * W * P)
            nc.scalar.activation(psqt[:], pT[:, sl], ACTF.Square)
            nc.tensor.matmul(
                pm_ps[:], ones64[:], pT[:, sl], start=(r == 0), stop=(r == NRUN - 1)
            )
            nc.tensor.matmul(
                psq_ps[:], ones64[:], psqt[:], start=(r == 0), stop=(r == NRUN - 1)
            )
        psmall = _ppool.tile([1, 3 * W * P], F32)
        pm_sb = psmall[:, 0 : W * P]
        pinv_sb = psmall[:, W * P : 2 * W * P]
        pt2 = psmall[:, 2 * W * P : 3 * W * P]
        nc.scalar.copy(out=pm_sb, in_=pm_ps[:])
        nc.vector.tensor_mul(pt2, pm_sb, pm_sb)
        nc.vector.scalar_tensor_tensor(
            out=pt2, in0=pt2, scalar=-1.0 / C, in1=psq_ps[:], op0=ALU.mult, op1=ALU.add
        )
        nc.scalar.activation(pt2, pt2, ACTF.Ln)
        nc.scalar.activation(pinv_sb, pt2, ACTF.Exp, scale=-0.5)

        ones1 = _ppool.tile([1, 128], F32)
        nc.vector.memset(ones1[:], 1.0)
        pmB = _ppsC.tile([RC, W * P], F32, tag="pbb")
        pnB = _ppsC.tile([RC, W * P], F32, tag="pbb")
        nc.tensor.matmul(pmB[:], ones1[:, 0:RC], pm_sb, start=True, stop=True)
        nc.tensor.matmul(pnB[:], ones1[:, 0:RC], pinv_sb, start=True, stop=True)
        pnf = _ppool.tile([RC, P * W], F32)
        for r in range(NRUN):
            sl = slice(r * W * P, (r + 1) * W * P)
            nc.vector.scalar_tensor_tensor(
                out=pnf[:], in0=pmB[:], scalar=-1.0 / C, in1=pT[:, sl],
                op0=ALU.mult, op1=ALU.add,
            )
            nc.vector.tensor_mul(pnf[:], pnf[:], pnB[:])
            nc.scalar.copy(
                out=pn_bf[:, sl].rearrange("c (w j) -> c w j", j=P),
                in_=pnf.rearrange("c (j w) -> c w j", j=P),
            )

        # spn = sum_c pn -> broadcast to 128 partitions
        spn_ps = _ppsB.tile([1, W * P], F32, tag="pmps")
        for r in range(NRUN):
            nc.tensor.matmul(
                spn_ps[:], ones64[:], pn_bf[:, r * W * P : (r + 1) * W * P],
                start=(r == 0), stop=(r == NRUN - 1),
            )
        spn_sb1 = pt2  # pt2 is dead once pinv_sb is computed
        nc.scalar.copy(out=spn_sb1, in_=spn_ps[:])
        spnB = _ppsC.tile([128, W * P], F32, tag="pbb")
        nc.tensor.matmul(spnB[:], ones1[:], spn_sb1, start=True, stop=True)
        nc.scalar.copy(out=spn_b[:], in_=spnB[:])



    # ---------------- query pools (PSUM pools created after preamble) ----
    qload = ctx.enter_context(tc.tile_pool(name="qload", bufs=2))
    qshare = ctx.enter_context(tc.tile_pool(name="qshare", bufs=1))
    qtile = ctx.enter_context(tc.tile_pool(name="qtile", bufs=1))
    qsmall = ctx.enter_context(tc.tile_pool(name="qsmall", bufs=1))
    qpsum = {}

    st = [dict() for _ in range(NT)]

    def tiles(t, name, shape, dtype, pool=qtile):
        if name not in st[t]:
            st[t][name] = pool.tile(
                shape, dtype, name=f"{name}{t}", tag=f"{name}{t}"
            )
        return st[t][name]

    def s1_load(t):
        qsl = slice(t * QT, (t + 1) * QT)
        for quarter in range(4):
            qraw = qload.tile([QT, CQ * S], F32, tag="qraw")
            c0 = quarter * CQ
            eng = nc.sync if quarter < 2 else nc.scalar
            eng.dma_start(
                out=qraw[:],
                in_=query[qsl, c0 : c0 + CQ].rearrange("q c h v -> q (c h v)"),
            )
            st[t][f"qraw{quarter}"] = qraw

    def s1_col(t):
        cwa = st[t]["cwa"] = qshare.tile([QT, P * C], BF16, name="cwa", tag="cwa")
        cwb = st[t]["cwb"] = qshare.tile([QT, P * C], BF16, name="cwb", tag="cwb")
        cwc = st[t]["cwc"] = qshare.tile([QT, P * C], BF16, name="cwc", tag="cwc")
        g, v = nc.gpsimd, nc.vector
        plans = [[g, v, g, g, v], [v, g, g, v, g], [g, v, g, g, v], [v, g, g, v, g]]
        for quarter in range(4):
            qraw = st[t].pop(f"qraw{quarter}")
            t5 = qload.tile([QT, CQ * P], BF16, tag="t5")
            xv = qraw.rearrange("q (c h v) -> q c h v", h=5, v=5)
            _col_stage(nc, (cwa, cwb, cwc, t5), xv, CQ, quarter * CQ, plans[quarter])

    def s2_row_norms(t):
        cwa = st[t].pop("cwa")
        cwb = st[t].pop("cwb")
        cwc = st[t].pop("cwc")
        qf = tiles(t, "qf", [QT, P * C], BF16)
        scr = qshare.tile([QT, 2 * C], BF16, name="scr", tag="scr")
        _row_stage(nc, qf, cwa, cwb, cwc, scr)

        sm = tiles(t, "sm", [QT, 8 * P], F32, pool=qsmall)
        msq = sm[:, P : 2 * P]
        dummy = scr[:, 0:C]
        for pi in range(P):
            qp = qf[:, pi * C : (pi + 1) * C]
            nc.vector.scalar_tensor_tensor(
                out=dummy, in0=qp, scalar=1.0, in1=qp, op0=ALU.mult, op1=ALU.mult,
                accum_out=msq[:, pi : pi + 1],
            )

    def s3_mm(t):
        qf = tiles(t, "qf", [QT, P * C], BF16)
        # qfT [64, (patch, run, q)]: chunk m = i*10+r, two 64-channel
        # transposes per PSUM tile, all operands partition-0 based
        qfT = tiles(t, "qfT", [RC, 50 * QT], BF16)
        for pr in range(25):
            tps = qpsum["tps"].tile([RC, 2 * QT], BF16, tag="tps")
            for h in range(2):
                m = pr * 2 + h
                nc.tensor.transpose(
                    tps[:, h * QT : (h + 1) * QT],
                    qf[:, m * RC : (m + 1) * RC], ident_bf[:],
                )
            dst = qfT[:, pr * 2 * QT : (pr * 2 + 2) * QT]
            if pr % 2 == 0:
                nc.scalar.copy(out=dst, in_=tps[:])
            else:
                nc.vector.tensor_copy(dst, tps[:])

        sim = tiles(t, "sim", [QT, W * S], F32)
        simv = sim.rearrange("q (w i j) -> q w i j", i=P, j=P)
        w1 = tiles(t, "w1", [QT, W * P], F32, pool=qsmall)
        sm = tiles(t, "sm", [QT, 8 * P], F32, pool=qsmall)
        msum = sm[:, 0:P]
        spnv = spn_b.rearrange("q (w j) -> q w j", j=P)
        mms = []
        for pi in range(P):
            mmw = qpsum["mm"].tile([QT, W * P + WM], F32, tag="mm", bufs=5)
            mm = mmw[:, 0 : W * P]
            mw = mmw[:, W * P : W * P + WM]
            def lhs_of(r):
                m = pi * NRUN + r
                return qfT[:, m * QT : (m + 1) * QT]

            for r in range(NRUN):
                pn_s = pn_bf[:, r * W * P : (r + 1) * W * P]
                nc.tensor.matmul(
                    mm, lhs_of(r), pn_s, start=(r == 0), stop=(r == NRUN - 1)
                )
            for r in range(NRUN):
                wm_s = wm_bf[:, (r * P + pi) * WM : (r * P + pi + 1) * WM]
                nc.tensor.matmul(
                    mw, lhs_of(r), wm_s, start=(r == 0), stop=(r == NRUN - 1)
                )
            mms.append((mm, mw))
        # norms: nrm2 = msq - msum^2/C, invn = exp(-.5 ln), minvn = -msum/C*invn
        msq = sm[:, P : 2 * P]
        nrm2 = sm[:, 2 * P : 3 * P]
        invn = sm[:, 3 * P : 4 * P]
        minvn = sm[:, 4 * P : 5 * P]
        for pi in range(P):
            mm, mw = mms[pi]
            nc.scalar.copy(out=w1[:, pi : (W - 1) * P + pi + 1 : P], in_=mw[:, 0:W])
            nc.vector.tensor_copy(msum[:, pi : pi + 1], mw[:, W : W + 1])
        nc.vector.tensor_mul(nrm2, msum, msum)
        nc.vector.scalar_tensor_tensor(
            out=nrm2, in0=nrm2, scalar=-1.0 / C, in1=msq, op0=ALU.mult, op1=ALU.add
        )
        nc.scalar.activation(nrm2, nrm2, ACTF.Ln)
        nc.scalar.activation(invn, nrm2, ACTF.Exp, scale=-0.5)
        nc.vector.scalar_tensor_tensor(
            out=minvn, in0=msum, scalar=-1.0 / C, in1=invn, op0=ALU.mult, op1=ALU.mult
        )
        # marginals first: frees w1 to double as the scale scratch below
        A = tiles(t, "A", [QT, W * P], F32, pool=qsmall)
        inva = tiles(t, "inva", [QT, W * P], F32, pool=qsmall)
        Ssum = tiles(t, "Ssum", [QT, W], F32, pool=qsmall)
        nc.vector.tensor_scalar(
            out=A[:], in0=w1[:], scalar1=0.0, scalar2=0.00101,
            op0=ALU.max, op1=ALU.add,
        )
        nc.vector.tensor_reduce(
            out=Ssum[:], in_=A.rearrange("q (w p) -> q w p", p=P), axis=AX.X,
            op=ALU.add,
        )
        nc.scalar.activation(inva[:], A[:], ACTF.Ln)
        nc.scalar.activation(inva[:], inva[:], ACTF.Exp, scale=-1.0)
        invav = inva.rearrange("q (w p) -> q w p", p=P)
        nc.vector.tensor_mul(
            invav, invav, Ssum[:, :, None].broadcast_to([QT, W, P])
        )
        tmp = w1
        for pi in range(P):
            mm, mw = mms[pi]
            nc.scalar.activation(tmp[:], mm[:], ACTF.Copy, scale=invn[:, pi : pi + 1])
            nc.vector.scalar_tensor_tensor(
                out=simv[:, :, pi, :], in0=spnv, scalar=minvn[:, pi : pi + 1],
                in1=tmp.rearrange("q (w j) -> q w j", j=P),
                op0=ALU.mult, op1=ALU.add,
            )

    def s4_K(t):
        sim = tiles(t, "sim", [QT, W * S], F32)
        simv = sim.rearrange("q (w i j) -> q w i j", i=P, j=P)
        inva = tiles(t, "inva", [QT, W * P], F32, pool=qsmall)
        invav = inva.rearrange("q (w p) -> q w p", p=P)
        # REP [q,(w,a,b)] = inva[w,a] repeated over b; serves K1 (a=i) and
        # K2 (a=j). Packed bf16 write via Act stride-0 read.
        REP = tiles(t, "REP", [QT, S * W], BF16)
        nc.scalar.activation(
            REP.rearrange("q (w a b) -> q w a b", a=P, b=P),
            invav[:, :, :, None].broadcast_to([QT, W, P, P]),
            ACTF.Copy,
        )
        K1 = tiles(t, "K1", [QT, S * W], BF16)
        K2 = tiles(t, "K2", [QT, S * W], BF16)
        nc.scalar.activation(
            K1.rearrange("q (w i j) -> q w i j", i=P, j=P), simv,
            ACTF.Exp, scale=EXP_SCALE, bias=ebias[:],
        )
        nc.scalar.activation(
            K2.rearrange("q (w j i) -> q w j i", j=P, i=P),
            simv.transpose([0, 1, 3, 2]),
            ACTF.Exp, scale=EXP_SCALE, bias=ebias[:],
        )
        nc.vector.tensor_mul(K1[:], K1[:], REP[:])
        nc.vector.tensor_mul(K2[:], K2[:], REP[:])

    # sinkhorn: u/v replicated tensors written by Act Exp with stride-0 views;
    # the dead replication buffer of the OTHER side doubles as the product
    # scratch (T) each half-iteration. Work is split into w-halves so four
    # independent chains (2 tiles x 2 halves) keep DVE and Act filled.
    WH = W // 2

    def s5_half1(t, wh, first):
        K1 = tiles(t, "K1", [QT, S * W], BF16)
        su = tiles(t, "su", [QT, W * P], F32, pool=qsmall)
        VR = tiles(t, "VR", [QT, S * W], BF16)  # v rep: [q,(w,i,j)] = v[w,j]
        UR = tiles(t, "UR", [QT, S * W], BF16)  # u rep: [q,(w,j,i)] = u[w,i]
        ws = slice(wh * WH * S, (wh + 1) * WH * S)
        ws2 = slice(wh * WH * P, (wh + 1) * WH * P)
        if first:
            red_in = K1[:, ws]
        else:
            nc.vector.tensor_mul(UR[:, ws], K1[:, ws], VR[:, ws])
            red_in = UR[:, ws]
        nc.vector.tensor_reduce(
            out=su[:, ws2], in_=red_in.rearrange("q (x j) -> q x j", j=P),
            axis=AX.X, op=ALU.add,
        )
        nc.scalar.activation(su[:, ws2], su[:, ws2], ACTF.Ln)
        # UR[q,w,j,i] = exp(-lt[w,i]) : stride-0 middle j, packed inner i
        suv = su[:, ws2].rearrange("q (w i) -> q w i", i=P)
        nc.scalar.activation(
            UR[:, ws].rearrange("q (w j i) -> q w j i", j=P, i=P),
            suv[:, :, None, :].broadcast_to([QT, WH, P, P]),
            ACTF.Exp, scale=-1.0,
        )

    def s5_half2(t, wh):
        K2 = tiles(t, "K2", [QT, S * W], BF16)
        sv = tiles(t, "sv", [QT, W * P], F32, pool=qsmall)
        UR = tiles(t, "UR", [QT, S * W], BF16)
        VR = tiles(t, "VR", [QT, S * W], BF16)
        ws = slice(wh * WH * S, (wh + 1) * WH * S)
        ws2 = slice(wh * WH * P, (wh + 1) * WH * P)
        nc.vector.tensor_mul(VR[:, ws], K2[:, ws], UR[:, ws])
        nc.vector.tensor_reduce(
            out=sv[:, ws2], in_=VR[:, ws].rearrange("q (x i) -> q x i", i=P),
            axis=AX.X, op=ALU.add,
        )
        nc.scalar.activation(sv[:, ws2], sv[:, ws2], ACTF.Ln)
        svv = sv[:, ws2].rearrange("q (w j) -> q w j", j=P)
        nc.scalar.activation(
            VR[:, ws].rearrange("q (w i j) -> q w i j", i=P, j=P),
            svv[:, :, None, :].broadcast_to([QT, WH, P, P]),
            ACTF.Exp, scale=-1.0,
        )

    def s6_final(t):
        qsl = slice(t * QT, (t + 1) * QT)
        sim = tiles(t, "sim", [QT, W * S], F32)
        K1 = tiles(t, "K1", [QT, S * W], BF16)   # reused as K0 buffer
        K2 = tiles(t, "K2", [QT, S * W], BF16)   # reused as simb
        REP = tiles(t, "REP", [QT, S * W], BF16)  # reused as u_i rep (w,i,j)
        UR = tiles(t, "UR", [QT, S * W], BF16)    # product scratch
        VR = tiles(t, "VR", [QT, S * W], BF16)
        su = tiles(t, "su", [QT, W * P], F32, pool=qsmall)
        nc.scalar.activation(K1[:], sim[:], ACTF.Exp, scale=EXP_SCALE, bias=ebias[:])
        nc.scalar.copy(out=K2[:], in_=sim[:])
        # REP[q,w,i,j] = exp(-lt_u[w,i]) bcast over inner j (su still holds ln)
        suv = su.rearrange("q (w i) -> q w i", i=P)
        nc.scalar.activation(
            REP.rearrange("q (w i j) -> q w i j", i=P, j=P),
            suv[:, :, :, None].broadcast_to([QT, W, P, P]),
            ACTF.Exp, scale=-1.0,
        )
        nc.vector.tensor_mul(UR[:], K1[:], K2[:])
        nc.vector.tensor_mul(UR[:], UR[:], REP[:])
        nc.vector.tensor_mul(UR[:], UR[:], VR[:])
        logits = tiles(t, "logits", [QT, W], F32, pool=qsmall)
        nc.vector.tensor_reduce(
            out=logits[:], in_=UR.rearrange("q (w s) -> q w s", s=S), axis=AX.X,
            op=ALU.add,
        )
        nc.scalar.mul(logits[:], logits[:], FINAL_SCALE)
        nc.sync.dma_start(out=out[qsl, :], in_=logits[:])

    # ---- emission ----
    pctx = ExitStack()
    _ppool = pctx.enter_context(tc.tile_pool(name="ppool", bufs=1))
    _pload = pctx.enter_context(tc.tile_pool(name="pload", bufs=2))
    _ppsA = pctx.enter_context(tc.tile_pool(name="ppsA", bufs=2, space="PSUM"))
    _ppsB = pctx.enter_context(tc.tile_pool(name="ppsB", bufs=3, space="PSUM"))
    _ppsC = pctx.enter_context(tc.tile_pool(name="ppsC", bufs=2, space="PSUM"))

    emit_preamble_load()
    s1_load(0)
    s1_col(0)
    emit_preamble_compute()
    pctx.close()
    qpsum["tps"] = ctx.enter_context(tc.tile_pool(name="tps", bufs=2, space="PSUM"))
    qpsum["mm"] = ctx.enter_context(tc.tile_pool(name="mmp", bufs=5, space="PSUM"))
    s1_load(1)
    s2_row_norms(0)
    s3_mm(0)
    s1_col(1)
    s4_K(0)
    # tile 0's first sinkhorn iteration overlaps tile 1's matmul phase
    for wh in range(2):
        s5_half1(0, wh, first=True)
    s2_row_norms(1)
    for wh in range(2):
        s5_half2(0, wh)
    s3_mm(1)
    s4_K(1)
    for it in range(ITERS):
        for wh in range(2):
            if it > 0:
                s5_half1(0, wh, first=False)
            s5_half1(1, wh, first=(it == 0))
        for wh in range(2):
            if it > 0:
                s5_half2(0, wh)
            s5_half2(1, wh)
    for t in range(NT):
        s6_final(t)


_NC_CACHE = {}


def kernel(proto: np.ndarray, query: np.ndarray) -> np.ndarray:
    from concourse.bass_utils import run_bass_kernel_spmd

    if "nc" not in _NC_CACHE:
        _NC_CACHE["nc"] = build_bass()
    nc = _NC_CACHE["nc"]
    proto = np.ascontiguousarray(proto, dtype=np.float32)
    query = np.ascontiguousarray(query, dtype=np.float32)
    in_maps = [
        {"proto": proto, "query": query[i * QPC : (i + 1) * QPC]}
        for i in range(N_CORES)
    ]
    res = run_bass_kernel_spmd(nc, in_maps, core_ids=list(range(N_CORES)))
    return np.concatenate([r["out"] for r in res.results], axis=0)
